# revision 9
# baseline (speedup 1.0000x reference)
"""Multi-head causal attention (B=4, T=2048, C=1024, H=16, D=64) on 8 TRN2
NeuronCores via Bass/Tile.

Sharding: core c = 2*b_pair... core id = 2*bgroup + g where bgroup = batch index
(B=4 batches -> 4 pairs of cores), g in {0,1} selects 8 of the 16 heads.
Each core:
  - receives x^T for its batch [C, T], the w_qkv columns for its 8 heads
    [C, 3*512], w_out rows for its heads [512, C], and a bias (full b_out on
    g==0 cores, zeros on g==1 so the pair ReduceScatter adds it exactly once).
  - computes Q^T,K^T (transposed) and V (plain) projections, causal softmax
    attention for its 8 heads in a transposed dataflow (scores^T = K @ Q^T with
    keys on partitions, softmax denominator via a ones-augmented V column in
    the P@V matmul), the w_out partial product, and ReduceScatters the partial
    with its pair sibling so each core ends with half the output channels.
Host side: shard/transpose inputs, gather [512, T] halves, transpose back.
"""
import sys

for _p in ("/opt/trn_rl_repo", "/root/.axon_site/_ro/trn_rl_repo"):
    if _p not in sys.path:
        sys.path.append(_p)

import numpy as np
import concourse.bass as bass
import concourse.tile as tile
from concourse import bacc, mybir
from concourse.bass_utils import run_bass_kernel_spmd

F32 = mybir.dt.float32

B, T, C = 4, 2048, 1024
H, D = 16, 64
NCORES = 8
HL = 8            # local heads per core
CL = HL * D       # 512 local channels
TCH = 512         # t-chunk (query tile)
NJ = T // TCH     # 4 chunks
KT = C // 128     # 8 contraction tiles for projections
KO = CL // 128    # 4 contraction tiles for out-proj
NEG = -1e10


def build(collective=True, reps=1, ps_cfg=(1, 4, 2, 1), pp_bufs=6):
    nc = bacc.Bacc("TRN2", target_bir_lowering=False, debug=False,
                   num_devices=NCORES)
    x_t = nc.dram_tensor("x_t", [C, T], F32, kind="ExternalInput").ap()
    w_qkv = nc.dram_tensor("w_qkv", [C, 3 * CL], F32, kind="ExternalInput").ap()
    w_out = nc.dram_tensor("w_out", [CL, C], F32, kind="ExternalInput").ap()
    b_eff = nc.dram_tensor("b_eff", [C], F32, kind="ExternalInput").ap()
    out_half = nc.dram_tensor("out_half", [CL, T], F32, kind="ExternalOutput").ap()

    with tile.TileContext(nc) as tc:
        with (
            tc.tile_pool(name="consts", bufs=1) as consts,
            tc.tile_pool(name="weights", bufs=1) as weights,
            tc.tile_pool(name="kv", bufs=1) as kv,
            tc.tile_pool(name="xin", bufs=1) as xin,
            tc.tile_pool(name="qp", bufs=2) as qp,
            tc.tile_pool(name="pp", bufs=pp_bufs) as pp,
            tc.tile_pool(name="att", bufs=2) as att,
            tc.tile_pool(name="sm", bufs=2) as sm,
            tc.tile_pool(name="outp", bufs=2) as outp,
            tc.tile_pool(name="ps_proj", bufs=ps_cfg[0], space="PSUM") as ps_proj,
            tc.tile_pool(name="ps_s", bufs=ps_cfg[1], space="PSUM") as ps_s,
            tc.tile_pool(name="ps_pv", bufs=ps_cfg[2], space="PSUM") as ps_pv,
            tc.tile_pool(name="ps_o", bufs=ps_cfg[3], space="PSUM") as ps_o,
            tc.tile_pool(name="dram", bufs=2, space="DRAM") as dram,
        ):
            # ---- constants ----
            mask = consts.tile([128, 128], F32)
            nc.vector.memset(mask[:], 0.0)
            # keep 0 where f >= p (k <= q), else NEG
            nc.gpsimd.affine_select(
                out=mask[:], in_=mask[:], compare_op=mybir.AluOpType.is_ge,
                fill=NEG, base=0, pattern=[[1, 128]], channel_multiplier=-1,
            )
            b_sb = consts.tile([128, KT], F32)
            nc.sync.dma_start(b_sb[:], b_eff.rearrange("(mo p) -> p mo", p=128))

            # ---- weights ----
            w_t = weights.tile([128, KT, 3 * CL], F32)
            nc.sync.dma_start(w_t[:], w_qkv.rearrange("(kt p) n -> p kt n", p=128))
            wo_t = weights.tile([128, KO, C], F32)
            nc.sync.dma_start(wo_t[:], w_out.rearrange("(ko p) n -> p ko n", p=128))

            # ---- persistent K^T and (ones-augmented) V ----
            kt_t = kv.tile([128, KO, T], F32)        # K^T: rows = local c, cols = t
            v_t = kv.tile([128, T // 128, HL * 65], F32)  # V rows = t, 65 cols/head
            # ones columns (col 64 of each 65-wide head block)
            v_aug = v_t.rearrange("p tt (h e) -> p tt h e", e=65)
            nc.vector.memset(v_aug[:, :, :, 64:65], 1.0)

            x_r = x_t.rearrange("(kt p) t -> p kt t", p=128)

            for _rep in range(reps):
              for j in range(NJ):
                ts = slice(j * TCH, (j + 1) * TCH)
                # ---- load x^T chunk ----
                xc = xin.tile([128, KT, TCH], F32)
                nc.sync.dma_start(xc[:], x_r[:, :, ts])

                # ---- projections for this chunk ----
                qt_c = qp.tile([128, KO, TCH], F32)
                for m in range(KO):
                    psq = ps_proj.tile([128, TCH], F32, tag="proj")
                    for k in range(KT):
                        nc.tensor.matmul(
                            psq[:], w_t[:, k, 128 * m:128 * (m + 1)], xc[:, k, :],
                            start=(k == 0), stop=(k == KT - 1))
                    nc.vector.tensor_scalar_mul(qt_c[:, m, :], psq[:], float(D) ** -0.5)
                for m in range(KO):
                    psk = ps_proj.tile([128, TCH], F32, tag="proj")
                    for k in range(KT):
                        nc.tensor.matmul(
                            psk[:], w_t[:, k, CL + 128 * m:CL + 128 * (m + 1)],
                            xc[:, k, :], start=(k == 0), stop=(k == KT - 1))
                    nc.vector.tensor_copy(kt_t[:, m, ts], psk[:])
                for ttl in range(TCH // 128):
                    tt = j * (TCH // 128) + ttl
                    psv = ps_proj.tile([128, CL], F32, tag="proj")
                    for k in range(KT):
                        nc.tensor.matmul(
                            psv[:], xc[:, k, 128 * ttl:128 * (ttl + 1)],
                            w_t[:, k, 2 * CL:3 * CL],
                            start=(k == 0), stop=(k == KT - 1))
                    nc.vector.tensor_copy(
                        v_aug[:, tt, :, 0:64],
                        psv.rearrange("p (h d) -> p h d", h=HL))

                # ---- attention for this chunk ----
                # two heads of a pair interleaved: their K=64 score matmuls
                # sit in different PE row groups (base partitions 0 / 64) and
                # run concurrently when adjacent in the instruction stream.
                at_c = att.tile([128, KO, TCH], F32)
                for m in range(KO):
                    ha, hb = 2 * m, 2 * m + 1
                    pva = ps_pv.tile([65, TCH], F32, tag="pv")
                    pvb = ps_pv.tile([65, TCH], F32, tag="pv")
                    nkb = 4 * (j + 1)
                    for kb in range(nkb):
                        r = kb - 4 * j
                        off = 128 * max(r, 0)
                        ks = slice(128 * kb, 128 * (kb + 1))
                        spa = ps_s.tile([128, TCH], F32, tag="s")
                        spb = ps_s.tile([128, TCH], F32, tag="s")
                        nc.tensor.matmul(
                            spa[:, off:], kt_t[0:64, m, ks], qt_c[0:64, m, off:])
                        nc.tensor.matmul(
                            spb[:, off:], kt_t[64:128, m, ks], qt_c[64:128, m, off:])
                        if r >= 0:
                            nc.vector.tensor_add(
                                spa[:, off:off + 128], spa[:, off:off + 128], mask[:])
                            nc.vector.tensor_add(
                                spb[:, off:off + 128], spb[:, off:off + 128], mask[:])
                        pa = pp.tile([128, TCH], F32, tag="p")
                        pb = pp.tile([128, TCH], F32, tag="p")
                        nc.scalar.activation(
                            pa[:, off:], spa[:, off:], mybir.ActivationFunctionType.Exp)
                        nc.scalar.activation(
                            pb[:, off:], spb[:, off:], mybir.ActivationFunctionType.Exp)
                        nc.tensor.matmul(
                            pva[:, off:], v_t[:, kb, 65 * ha:65 * ha + 65],
                            pa[:, off:], start=(kb == 0), stop=(kb == nkb - 1))
                        nc.tensor.matmul(
                            pvb[:, off:], v_t[:, kb, 65 * hb:65 * hb + 65],
                            pb[:, off:], start=(kb == 0), stop=(kb == nkb - 1))
                    for half, pv in ((0, pva), (1, pvb)):
                        r0 = 64 * half
                        rc = sm.tile([1, TCH], F32, tag="rc")
                        nc.vector.reciprocal(rc[:], pv[64:65, :])
                        bc = sm.tile([64, TCH], F32, tag="bc")
                        nc.gpsimd.partition_broadcast(bc[:], rc[:])
                        nc.vector.tensor_mul(at_c[r0:r0 + 64, m, :], pv[0:64, :], bc[:])

                # ---- output projection partial for this chunk ----
                cc_in = dram.tile([C, TCH], F32)
                for mo in range(KT):
                    pso = ps_o.tile([128, TCH], F32)
                    for kb in range(KO):
                        nc.tensor.matmul(
                            pso[:], wo_t[:, kb, 128 * mo:128 * (mo + 1)],
                            at_c[:, kb, :], start=(kb == 0), stop=(kb == KO - 1))
                    ob = outp.tile([128, TCH], F32)
                    nc.vector.tensor_scalar_add(ob[:], pso[:], b_sb[:, mo:mo + 1])
                    nc.sync.dma_start(cc_in[128 * mo:128 * (mo + 1), :], ob[:])

                cc_out = dram.tile([CL, TCH], F32)
                if collective:
                    nc.gpsimd.collective_compute(
                        "ReduceScatter", mybir.AluOpType.add,
                        replica_groups=[[0, 1], [2, 3], [4, 5], [6, 7]],
                        ins=[cc_in.opt()], outs=[cc_out.opt()])
                else:
                    nc.sync.dma_start(cc_out[:], cc_in[0:CL, :])
                nc.sync.dma_start(out_half[:, ts], cc_out[:])

    nc.compile()
    return nc


_NC_CACHE = {}


def get_nc(collective=True, reps=1):
    key = (collective, reps)
    if key not in _NC_CACHE:
        _NC_CACHE[key] = build(collective, reps)
    return _NC_CACHE[key]


def make_in_maps(x, w_qkv, w_out, b_out):
    x = np.asarray(x, dtype=np.float32)
    w_qkv = np.asarray(w_qkv, dtype=np.float32)
    w_out = np.asarray(w_out, dtype=np.float32)
    b_out = np.asarray(b_out, dtype=np.float32)
    in_maps = []
    zeros_b = np.zeros_like(b_out)
    for c in range(NCORES):
        bi, g = c // 2, c % 2
        cols = slice(CL * g, CL * (g + 1))
        w_loc = np.ascontiguousarray(np.concatenate(
            [w_qkv[:, cols], w_qkv[:, C:][:, cols], w_qkv[:, 2 * C:][:, cols]],
            axis=1))
        in_maps.append({
            "x_t": np.ascontiguousarray(x[bi].T),
            "w_qkv": w_loc,
            "w_out": np.ascontiguousarray(w_out[CL * g:CL * (g + 1), :]),
            "b_eff": b_out if g == 0 else zeros_b,
        })
    return in_maps


def assemble(results):
    out = np.empty((B, T, C), dtype=np.float32)
    for bi in range(B):
        top = results[2 * bi]["out_half"]       # channels 0:512
        bot = results[2 * bi + 1]["out_half"]   # channels 512:1024
        out[bi] = np.concatenate([top, bot], axis=0).T
    return out


def kernel(x, w_qkv, w_out, b_out):
    nc = get_nc(collective=True)
    in_maps = make_in_maps(x, w_qkv, w_out, b_out)
    res = run_bass_kernel_spmd(nc, in_maps, list(range(NCORES)))
    return assemble(res.results)


if __name__ == "__main__":
    nc = build()
    print("instructions:", len(nc.inst_map))
